# revision 26
# baseline (speedup 1.0000x reference)
"""Trainium2 Bass kernel for nn_BaselineDistiller: grouped-expert MLP + MSE loss.

reference:
    h    = einsum('bne,neh->bnh', features, W1) + b1
    g    = gelu(h)                      # exact (erf) gelu
    pred = einsum('bnh,nhe->bne', g, W2) + b2
    out  = mean((pred - target)^2)

Strategy (8 NeuronCores, data-parallel over batch; ~153-157us on HW):
  The kernel is ScalarE(gelu)-bound: 131072 gelu elems/partition/core at
  1 elem/cycle/1.2GHz = 109us floor (+ ~150ns/op overhead). Everything else
  is arranged to keep the ACT engine gap-free and shrink head/tail:
  * All inputs are fp8 e4m3 (feat*8, targ*16 with b2 folded, W1*8, W2*16)
    -> DMA-in halves to ~19MB/core (~57us) and matmuls run at fp8 rate.
  * mm1 (h.T = W1c.T @ feat.T per 128-row chunk) writes PSUM; ACT applies
    gelu with per-chunk bias b1 and scale 1/64 in 4 FD-1024 ops/expert
    (each op is single-chunk so the per-partition bias is uniform), writing
    fp8 hact laid out [128, 4tiles, 2ktiles, 512] (k-tiles contiguous) so
    mm2 runs as ONE fp8 DoubleRow matmul per tile (K=256 at 1 col/cycle —
    2x FLOPs; measured same 216ns as a K=128 matmul).
  * pred tiles accumulate [W2-DR, -I @ (16*(targ-b2))] so PSUM ends holding
    16*diff; one DVE bn_stats per tile (the only DVE reduction needing a
    single PSUM read) yields per-256-group {n, mean, M2}; the host
    reconstructs sum(diff^2) = sum M2 + n*mean^2, dividing out the 16^2.
  * PSUM: ph pool (2 bufs x 2 banks) is a pure mm1->ACT ping-pong; pp pool
    (2 bufs x 2 banks) holds pred pairs. Each expert emits 4 symmetric
    sub-blocks [mm1 pair, ACT, mm2+bn of the PREVIOUS expert's tile k], so
    every cross-engine chain has ~0.5us of slack at the ACT cadence and the
    in-order PE never starves the ACT queue (measured <2us of ACT gaps
    after the DMA ramp; PE ~77%, DVE ~62% of the cadence).
  * A dummy gelu at t=0 pulls the 1.3us ACT table load into the DMA ramp;
    20 short junk matmuls bridge the ramp so the first real mm1 runs at
    full PE p-state; feat DMAs are prefetched 3 experts ahead and the first
    weight group is issued with the head so expert 1-2 mm1s never wait;
    stats ship in a 96/32 split so the tail only waits on the last expert.
    Typical HW time ~152-156us at nominal clocks (the chip DVFS-throttles
    some runs ~20%; the schedule stays gap-free either way).
"""

import contextlib
import ctypes
import json
import sys
import types

import ml_dtypes
import numpy as np

import concourse.bass as bass
import concourse.mybir as mybir
import concourse.tile as tile
from concourse import bass_utils
from concourse.bass import ts
from concourse.bass_utils import run_bass_kernel_spmd

B, NE, E, H = 16384, 32, 128, 256
C = 8              # cores
BS = B // C        # batch rows per core
BT = 512           # batch columns per matmul tile
NT = BS // BT      # 4
FP8 = mybir.dt.float8e4
F32 = mybir.dt.float32
DR = mybir.MatmulPerfMode.DoubleRow

S_X = 8.0          # feature scale into fp8
S_W1 = 8.0
S_W2 = 16.0        # also the target scale (so pred/targ match in PSUM)
ACT_SCALE = 1.0 / (S_X * S_W1)
TTR_SCALE = 1.0 / (S_W2 * S_W2)

# ---------------------------------------------------------------------------
# Environment shims (idempotent):
#  1. antenv.axon_hooks — the image's antenv lacks it; provide the NTFF
#     profile hook via ctypes so trace=True works when a caller requests it.
#  2. upload_artifacts — no bucket access in this container; keep local.
#  3. This walrus build rejects instructions with >1 sync-wait; split the
#     extra waits onto NoOps at BIR-serialization time.
# ---------------------------------------------------------------------------
_AXON_SO = "/opt/axon/libaxon_pjrt.so"


def _make_ntff_hook(so_path):
    try:
        lib = ctypes.CDLL(so_path)
    except OSError:
        return None
    if not hasattr(lib, "axon_start_nrt_profile"):
        return None
    lib.axon_start_nrt_profile.argtypes = [ctypes.POINTER(ctypes.c_int64), ctypes.c_size_t]
    lib.axon_start_nrt_profile.restype = ctypes.c_int64
    lib.axon_stop_nrt_profile.argtypes = [ctypes.c_char_p]
    lib.axon_stop_nrt_profile.restype = ctypes.c_int64

    @contextlib.contextmanager
    def _hook(output_dir, device_ids):
        import jax

        jax.devices()
        if device_ids:
            ids = (ctypes.c_int64 * len(device_ids))(*device_ids)
            rc = lib.axon_start_nrt_profile(ids, len(device_ids))
        else:
            rc = lib.axon_start_nrt_profile(None, 0)
        if rc != 0:
            raise RuntimeError(f"axon_start_nrt_profile rc={rc}")
        try:
            yield
        finally:
            n = lib.axon_stop_nrt_profile(str(output_dir).encode())
            print(f"profile: {n} file(s) written to {output_dir}", file=sys.stderr)

    return _hook


if "antenv.axon_hooks" not in sys.modules:
    _mod = types.ModuleType("antenv.axon_hooks")
    _the_hook = _make_ntff_hook(_AXON_SO)
    _mod.get_axon_ntff_profile_hook = lambda: _the_hook
    sys.modules["antenv.axon_hooks"] = _mod

bass_utils.upload_artifacts = lambda tmpdir: str(tmpdir)

_MAXW = 1
if not getattr(bass.Bass, "_wait_split_installed", False):
    _orig_to_json_bytes = bass.Bass.to_json_bytes

    def _split_sync_waits(self, *a, **kw):
        bir = json.loads(_orig_to_json_bytes(self, *a, **kw))
        for fn in bir.get("functions", []):
            for blk in fn.get("blocks", []):
                new_insts = []
                for inst in blk.get("instructions", []):
                    si = inst.get("sync_info") or {}
                    waits = si.get("on_wait") or []
                    if len(waits) > _MAXW:
                        extra, keep = waits[:-_MAXW], waits[-_MAXW:]
                        for k in range(0, len(extra), _MAXW):
                            new_insts.append({
                                "debug": inst.get("debug", 0),
                                "engine": inst["engine"],
                                "ins": [], "outs": [],
                                "name": f"{inst['name']}_wsplit{k}",
                                "opcode": "NoOp",
                                "sync_info": {"on_update": [],
                                              "on_wait": extra[k:k + _MAXW]},
                            })
                        si["on_wait"] = keep
                    new_insts.append(inst)
                blk["instructions"] = new_insts
        return json.dumps(bir).encode()

    bass.Bass.to_json_bytes = _split_sync_waits
    bass.Bass._wait_split_installed = True


# ---------------------------------------------------------------------------
# Device kernel
# ---------------------------------------------------------------------------
def _build_nc():
    nc = bass.Bass("TRN2", target_bir_lowering=False, debug=False)
    featd = nc.declare_dram_parameter("featT", [NE, E, BS], FP8, isOutput=False)
    targd = nc.declare_dram_parameter("targT", [NE, E, BS], FP8, isOutput=False)
    w1d = nc.declare_dram_parameter("w1", [E, NE, H], FP8, isOutput=False)
    w2d = nc.declare_dram_parameter("w2", [128, NE, 2, E], FP8, isOutput=False)
    # head = [negI | b1(f32-as-bytes) | W1[e0] | W2[e0]] so one DMA unblocks
    # expert 0 entirely.
    headd = nc.declare_dram_parameter("head", [128, 7, 128], FP8, isOutput=False)
    statsd = nc.declare_dram_parameter("stats", [128, 4 * NE, 6], F32, isOutput=True)

    GE = 8                     # experts per weight-DMA group
    NG = NE // GE

    with tile.TileContext(nc) as tc, contextlib.ExitStack() as ctx:
        wpool = ctx.enter_context(tc.tile_pool(name="weights", bufs=1))
        iopool = ctx.enter_context(tc.tile_pool(name="io", bufs=4))
        hpool = ctx.enter_context(tc.tile_pool(name="hact", bufs=2))
        php = ctx.enter_context(tc.tile_pool(name="ph", bufs=2, space="PSUM"))
        ppp = ctx.enter_context(tc.tile_pool(name="pp", bufs=2, space="PSUM"))

        head_sb = wpool.tile([128, 7, 128], FP8)
        negi_sb = head_sb[:, 0, :]
        b1f = head_sb[:, 1:3, :].bitcast(F32)        # [128, 2, 32] (p, c, n)
        w1e0 = head_sb[:, 3:5, :]                    # [128, 2, 128] (p, c, m)
        w2e0 = head_sb[:, 5:7, :]                    # [128, 2, 128] DR lhsT
        w1_sb = wpool.tile([E, NE, H], FP8)          # [128, 32, 256]
        w2_sb = wpool.tile([128, NE, 2, E], FP8)
        stats_sb = wpool.tile([128, 4 * NE, 6], F32)
        warm_sb = wpool.tile([128, 1], F32)

        # head rides the Activation engine's DGE queue so its descriptor
        # generation runs in parallel with feat0's on SP (ACT is idle here);
        # the gelu table then loads during the DMA ramp.
        nc.scalar.dma_start(out=head_sb[:], in_=headd[:])
        nc.gpsimd.memset(warm_sb[:], 0.0)
        nc.scalar.activation(warm_sb[:], warm_sb[:],
                             mybir.ActivationFunctionType.Gelu)

        def w1ap(n, c):
            return w1e0[:, c, :] if n == 0 else w1_sb[:, n, ts(c, 128)]

        def w2ap(n):
            return w2e0 if n == 0 else w2_sb[:, n, :, :]

        # flush tile t of the previous expert: mm2 (DoubleRow) + negI + bn
        def flush_tile(pend, t, pair_tile):
            n, hact, targ_sb = pend
            j = t % 2
            nc.tensor.matmul(pair_tile[:, j, :], lhsT=w2ap(n),
                             rhs=hact[:, t, :, :],
                             start=True, stop=False, perf_mode=DR)
            nc.tensor.matmul(pair_tile[:, j, :], lhsT=negi_sb,
                             rhs=targ_sb[:, ts(t, BT)],
                             start=False, stop=True)
            nc.vector.bn_stats(out=stats_sb[:, 4 * n + t, :],
                               in_=pair_tile[:, j, :])

        pending = None
        feat_tiles = {}

        def fetch_feat(n):
            if n < NE and n not in feat_tiles:
                f = iopool.tile([E, BS], FP8, tag="feat", name="feat_sb")
                nc.sync.dma_start(out=f[:], in_=featd[n])
                feat_tiles[n] = f

        f0 = iopool.tile([E, BS], FP8, tag="feat", name="feat_sb")
        nc.sync.dma_start(out=f0[:, 0:BS // 2], in_=featd[0][:, 0:BS // 2])
        nc.sync.dma_start(out=f0[:, BS // 2:], in_=featd[0][:, BS // 2:])
        feat_tiles[0] = f0
        nc.sync.dma_start(out=w1_sb[:, ts(0, GE), :], in_=w1d[:, ts(0, GE), :])
        # PE warm-up: keep the tensor engine busy through the DMA ramp so the
        # first real matmuls run at full p-state instead of cold-start speed.
        junk_sb = wpool.tile([128, 512], FP8)
        nc.gpsimd.memset(junk_sb[:], 0.0)
        warm_ps = php.tile([128, 2, BT], F32, name="warmps", tag="ph")
        for i in range(20):
            nc.tensor.matmul(warm_ps[:, i % 2, 0:256], lhsT=junk_sb[:, 0:128],
                             rhs=junk_sb[:, 0:256], start=True, stop=True)
        for n in range(NE):
            fetch_feat(n)
            feat_sb = feat_tiles.pop(n)
            fetch_feat(n + 1)
            fetch_feat(n + 2)
            fetch_feat(n + 3)
            targ_sb = iopool.tile([E, BS], FP8, tag="targ")
            nc.sync.dma_start(out=targ_sb[:], in_=targd[n])
            if n < 2 * NG - 1:
                g, which = divmod(n + 1, 2)
                if which == 0:
                    nc.sync.dma_start(out=w1_sb[:, ts(g, GE), :],
                                      in_=w1d[:, ts(g, GE), :])
                else:
                    nc.sync.dma_start(out=w2_sb[:, ts(g, GE), :, :],
                                      in_=w2d[:, ts(g, GE), :, :])
            if n == 25:
                # first 3/4 of the stats is final; overlap the store
                nc.sync.dma_start(out=statsd[:, 0:96, :],
                                  in_=stats_sb[:, 0:96, :])

            hact = hpool.tile([128, NT, 2, BT], FP8)
            p01 = p23 = None
            for g in range(4):
                c, pr = divmod(g, 2)        # chunk, tile-pair of this group
                ph_t = php.tile([128, 2, BT], F32, name=f"ph{g}", tag="ph")
                for i in range(2):
                    nc.tensor.matmul(ph_t[:, i, :], lhsT=w1ap(n, c),
                                     rhs=feat_sb[:, ts(2 * pr + i, BT)],
                                     start=True, stop=True)
                nc.scalar.activation(hact[:, 2 * pr:2 * pr + 2, c, :],
                                     ph_t[:, :, :],
                                     mybir.ActivationFunctionType.Gelu,
                                     bias=b1f[:, c, n:n + 1], scale=ACT_SCALE)
                if pending is not None:
                    if g == 0:
                        p01p = ppp.tile([128, 2, BT], F32, name="p01p", tag="pp")
                        flush_tile(pending, 0, p01p)
                    elif g == 1:
                        flush_tile(pending, 1, p01p)
                    elif g == 2:
                        p23p = ppp.tile([128, 2, BT], F32, name="p23p", tag="pp")
                        flush_tile(pending, 2, p23p)
                    else:
                        flush_tile(pending, 3, p23p)
            pending = (n, hact, targ_sb)
        pf01 = ppp.tile([128, 2, BT], F32, name="pf01", tag="pp")
        for t in (0, 1):
            flush_tile(pending, t, pf01)
        pf23 = ppp.tile([128, 2, BT], F32, name="pf23", tag="pp")
        for t in (2, 3):
            flush_tile(pending, t, pf23)
        nc.sync.dma_start(out=statsd[:, 96:, :], in_=stats_sb[:, 96:, :])
    return nc


LAST_RESULTS = None


def kernel(features, target_features, W1, b1, W2, b2):
    global LAST_RESULTS
    f8 = ml_dtypes.float8_e4m3
    features = np.asarray(features)
    target_features = np.asarray(target_features)
    W1 = np.asarray(W1)
    b1 = np.asarray(b1)
    W2 = np.asarray(W2)
    b2 = np.asarray(b2)

    feat4 = (features * S_X).reshape(C, BS, NE, E).transpose(0, 2, 3, 1).astype(f8)
    targ4 = ((target_features - b2[None]) * S_W2).reshape(C, BS, NE, E) \
        .transpose(0, 2, 3, 1).astype(f8)
    w1h = (W1 * S_W1).transpose(1, 0, 2).astype(f8)                  # [E, NE, H]
    w2h = (W2 * S_W2).reshape(NE, 2, 128, E).transpose(2, 0, 1, 3).astype(f8)
    b1h = np.ascontiguousarray(
        b1.reshape(NE, 2, 128).transpose(2, 1, 0).astype(np.float32))  # [p, c, n]

    negi = (-np.eye(128)).astype(f8)
    head = np.ascontiguousarray(np.concatenate(
        [negi.view(np.uint8),
         b1h.view(np.uint8).reshape(128, 256),
         np.ascontiguousarray(w1h[:, 0, :]).view(np.uint8),
         np.ascontiguousarray(w2h[:, 0, :, :]).reshape(128, 256).view(np.uint8)],
        axis=1)).view(f8).reshape(128, 7, 128)

    nc = _build_nc()
    in_maps = [
        {"featT": np.ascontiguousarray(feat4[c]),
         "targT": np.ascontiguousarray(targ4[c]),
         "w1": w1h, "w2": w2h, "head": head}
        for c in range(C)
    ]
    res = run_bass_kernel_spmd(nc, in_maps, list(range(C)))
    LAST_RESULTS = res
    # stats[p, slot] = [n0, mean0, M2_0, n1, mean1, M2_1] over the two
    # 256-halves of 16*diff; sum(diff^2) = (M2 + n*mean^2) / 256.
    total = 0.0
    for r in res.results:
        st = r["stats"].astype(np.float64)
        total += (st[..., 2] + st[..., 0] * st[..., 1] ** 2
                  + st[..., 5] + st[..., 3] * st[..., 4] ** 2).sum()
    return np.array(total * TTR_SCALE / (B * NE * E), dtype=np.float32)


# revision 27
# speedup vs baseline: 1.0019x; 1.0019x over previous
"""Trainium2 Bass kernel for nn_BaselineDistiller: grouped-expert MLP + MSE loss.

reference:
    h    = einsum('bne,neh->bnh', features, W1) + b1
    g    = gelu(h)                      # exact (erf) gelu
    pred = einsum('bnh,nhe->bne', g, W2) + b2
    out  = mean((pred - target)^2)

Strategy (8 NeuronCores, data-parallel over batch; ~153-157us on HW):
  The kernel is ScalarE(gelu)-bound: 131072 gelu elems/partition/core at
  1 elem/cycle/1.2GHz = 109us floor (+ ~150ns/op overhead). Everything else
  is arranged to keep the ACT engine gap-free and shrink head/tail:
  * All inputs are fp8 e4m3 (feat*8, targ*16 with b2 folded, W1*8, W2*16)
    -> DMA-in halves to ~19MB/core (~57us) and matmuls run at fp8 rate.
  * mm1 (h.T = W1c.T @ feat.T per 128-row chunk) writes PSUM; ACT applies
    gelu with per-chunk bias b1 and scale 1/64 in 4 FD-1024 ops/expert
    (each op is single-chunk so the per-partition bias is uniform), writing
    fp8 hact laid out [128, 4tiles, 2ktiles, 512] (k-tiles contiguous) so
    mm2 runs as ONE fp8 DoubleRow matmul per tile (K=256 at 1 col/cycle —
    2x FLOPs; measured same 216ns as a K=128 matmul).
  * pred tiles accumulate [W2-DR, -I @ (16*(targ-b2))] so PSUM ends holding
    16*diff; one DVE bn_stats per tile (the only DVE reduction needing a
    single PSUM read) yields per-256-group {n, mean, M2}; the host
    reconstructs sum(diff^2) = sum M2 + n*mean^2, dividing out the 16^2.
  * PSUM: ph pool (2 bufs x 2 banks) is a pure mm1->ACT ping-pong; pp pool
    (2 bufs x 2 banks) holds pred pairs. Each expert emits 4 symmetric
    sub-blocks [mm1 pair, ACT, mm2+bn of the PREVIOUS expert's tile k], so
    every cross-engine chain has ~0.5us of slack at the ACT cadence and the
    in-order PE never starves the ACT queue (measured <2us of ACT gaps
    after the DMA ramp; PE ~77%, DVE ~62% of the cadence).
  * A dummy gelu at t=0 pulls the 1.3us ACT table load into the DMA ramp;
    20 short junk matmuls bridge the ramp so the first real mm1 runs at
    full PE p-state; feat DMAs are prefetched 3 experts ahead and the first
    weight group is issued with the head so expert 1-2 mm1s never wait;
    stats ship in a 96/32 split so the tail only waits on the last expert.
    Typical HW time ~152-156us at nominal clocks (the chip DVFS-throttles
    some runs ~20%; the schedule stays gap-free either way).
"""

import contextlib
import ctypes
import json
import sys
import types

import ml_dtypes
import numpy as np

import concourse.bass as bass
import concourse.mybir as mybir
import concourse.tile as tile
from concourse import bass_utils
from concourse.bass import ts
from concourse.bass_utils import run_bass_kernel_spmd

B, NE, E, H = 16384, 32, 128, 256
C = 8              # cores
BS = B // C        # batch rows per core
BT = 512           # batch columns per matmul tile
NT = BS // BT      # 4
FP8 = mybir.dt.float8e4
F32 = mybir.dt.float32
DR = mybir.MatmulPerfMode.DoubleRow

S_X = 8.0          # feature scale into fp8
S_W1 = 8.0
S_W2 = 16.0        # also the target scale (so pred/targ match in PSUM)
ACT_SCALE = 1.0 / (S_X * S_W1)
TTR_SCALE = 1.0 / (S_W2 * S_W2)

# ---------------------------------------------------------------------------
# Environment shims (idempotent):
#  1. antenv.axon_hooks — the image's antenv lacks it; provide the NTFF
#     profile hook via ctypes so trace=True works when a caller requests it.
#  2. upload_artifacts — no bucket access in this container; keep local.
#  3. This walrus build rejects instructions with >1 sync-wait; split the
#     extra waits onto NoOps at BIR-serialization time.
# ---------------------------------------------------------------------------
_AXON_SO = "/opt/axon/libaxon_pjrt.so"


def _make_ntff_hook(so_path):
    try:
        lib = ctypes.CDLL(so_path)
    except OSError:
        return None
    if not hasattr(lib, "axon_start_nrt_profile"):
        return None
    lib.axon_start_nrt_profile.argtypes = [ctypes.POINTER(ctypes.c_int64), ctypes.c_size_t]
    lib.axon_start_nrt_profile.restype = ctypes.c_int64
    lib.axon_stop_nrt_profile.argtypes = [ctypes.c_char_p]
    lib.axon_stop_nrt_profile.restype = ctypes.c_int64

    @contextlib.contextmanager
    def _hook(output_dir, device_ids):
        import jax

        jax.devices()
        if device_ids:
            ids = (ctypes.c_int64 * len(device_ids))(*device_ids)
            rc = lib.axon_start_nrt_profile(ids, len(device_ids))
        else:
            rc = lib.axon_start_nrt_profile(None, 0)
        if rc != 0:
            raise RuntimeError(f"axon_start_nrt_profile rc={rc}")
        try:
            yield
        finally:
            n = lib.axon_stop_nrt_profile(str(output_dir).encode())
            print(f"profile: {n} file(s) written to {output_dir}", file=sys.stderr)

    return _hook


if "antenv.axon_hooks" not in sys.modules:
    _mod = types.ModuleType("antenv.axon_hooks")
    _the_hook = _make_ntff_hook(_AXON_SO)
    _mod.get_axon_ntff_profile_hook = lambda: _the_hook
    sys.modules["antenv.axon_hooks"] = _mod

bass_utils.upload_artifacts = lambda tmpdir: str(tmpdir)

_MAXW = 1
if not getattr(bass.Bass, "_wait_split_installed", False):
    _orig_to_json_bytes = bass.Bass.to_json_bytes

    def _split_sync_waits(self, *a, **kw):
        bir = json.loads(_orig_to_json_bytes(self, *a, **kw))
        for fn in bir.get("functions", []):
            for blk in fn.get("blocks", []):
                new_insts = []
                for inst in blk.get("instructions", []):
                    si = inst.get("sync_info") or {}
                    waits = si.get("on_wait") or []
                    if len(waits) > _MAXW:
                        extra, keep = waits[:-_MAXW], waits[-_MAXW:]
                        for k in range(0, len(extra), _MAXW):
                            new_insts.append({
                                "debug": inst.get("debug", 0),
                                "engine": inst["engine"],
                                "ins": [], "outs": [],
                                "name": f"{inst['name']}_wsplit{k}",
                                "opcode": "NoOp",
                                "sync_info": {"on_update": [],
                                              "on_wait": extra[k:k + _MAXW]},
                            })
                        si["on_wait"] = keep
                    new_insts.append(inst)
                blk["instructions"] = new_insts
        return json.dumps(bir).encode()

    bass.Bass.to_json_bytes = _split_sync_waits
    bass.Bass._wait_split_installed = True


# ---------------------------------------------------------------------------
# Device kernel
# ---------------------------------------------------------------------------
def _build_nc():
    nc = bass.Bass("TRN2", target_bir_lowering=False, debug=False)
    featd = nc.declare_dram_parameter("featT", [NE, E, BS], FP8, isOutput=False)
    targd = nc.declare_dram_parameter("targT", [NE, E, BS], FP8, isOutput=False)
    w1d = nc.declare_dram_parameter("w1", [E, NE, H], FP8, isOutput=False)
    w2d = nc.declare_dram_parameter("w2", [128, NE, 2, E], FP8, isOutput=False)
    # head = [negI | b1(f32-as-bytes) | W1[e0] | W2[e0]] so one DMA unblocks
    # expert 0 entirely.
    headd = nc.declare_dram_parameter("head", [128, 7, 128], FP8, isOutput=False)
    statsd = nc.declare_dram_parameter("stats", [128, 4 * NE, 6], F32, isOutput=True)

    GE = 8                     # experts per weight-DMA group
    NG = NE // GE

    with tile.TileContext(nc) as tc, contextlib.ExitStack() as ctx:
        wpool = ctx.enter_context(tc.tile_pool(name="weights", bufs=1))
        iopool = ctx.enter_context(tc.tile_pool(name="io", bufs=4))
        hpool = ctx.enter_context(tc.tile_pool(name="hact", bufs=2))
        php = ctx.enter_context(tc.tile_pool(name="ph", bufs=2, space="PSUM"))
        ppp = ctx.enter_context(tc.tile_pool(name="pp", bufs=2, space="PSUM"))

        head_sb = wpool.tile([128, 7, 128], FP8)
        negi_sb = head_sb[:, 0, :]
        b1f = head_sb[:, 1:3, :].bitcast(F32)        # [128, 2, 32] (p, c, n)
        w1e0 = head_sb[:, 3:5, :]                    # [128, 2, 128] (p, c, m)
        w2e0 = head_sb[:, 5:7, :]                    # [128, 2, 128] DR lhsT
        w1_sb = wpool.tile([E, NE, H], FP8)          # [128, 32, 256]
        w2_sb = wpool.tile([128, NE, 2, E], FP8)
        stats_sb = wpool.tile([128, 4 * NE, 6], F32)
        warm_sb = wpool.tile([128, 1], F32)

        # Load the gelu table during the DMA ramp instead of right before the
        # first real activation.
        nc.gpsimd.memset(warm_sb[:], 0.0)
        nc.scalar.activation(warm_sb[:], warm_sb[:],
                             mybir.ActivationFunctionType.Gelu)

        def w1ap(n, c):
            return w1e0[:, c, :] if n == 0 else w1_sb[:, n, ts(c, 128)]

        def w2ap(n):
            return w2e0 if n == 0 else w2_sb[:, n, :, :]

        # flush tile t of the previous expert: mm2 (DoubleRow) + negI + bn
        def flush_tile(pend, t, pair_tile):
            n, hact, targ_sb = pend
            j = t % 2
            nc.tensor.matmul(pair_tile[:, j, :], lhsT=w2ap(n),
                             rhs=hact[:, t, :, :],
                             start=True, stop=False, perf_mode=DR)
            nc.tensor.matmul(pair_tile[:, j, :], lhsT=negi_sb,
                             rhs=targ_sb[:, ts(t, BT)],
                             start=False, stop=True)
            nc.vector.bn_stats(out=stats_sb[:, 4 * n + t, :],
                               in_=pair_tile[:, j, :])

        pending = None
        feat_tiles = {}

        def fetch_feat(n):
            if n < NE and n not in feat_tiles:
                f = iopool.tile([E, BS], FP8, tag="feat", name="feat_sb")
                nc.sync.dma_start(out=f[:], in_=featd[n])
                feat_tiles[n] = f

        f0 = iopool.tile([E, BS], FP8, tag="feat", name="feat_sb")
        nc.sync.dma_start(out=f0[:, 0:BS // 2], in_=featd[0][:, 0:BS // 2])
        nc.sync.dma_start(out=head_sb[:], in_=headd[:])
        nc.sync.dma_start(out=f0[:, BS // 2:], in_=featd[0][:, BS // 2:])
        feat_tiles[0] = f0
        nc.sync.dma_start(out=w1_sb[:, ts(0, GE), :], in_=w1d[:, ts(0, GE), :])
        # PE warm-up: keep the tensor engine busy through the DMA ramp so the
        # first real matmuls run at full p-state instead of cold-start speed.
        junk_sb = wpool.tile([128, 512], FP8)
        nc.gpsimd.memset(junk_sb[:], 0.0)
        warm_ps = php.tile([128, 2, BT], F32, name="warmps", tag="ph")
        for i in range(20):
            nc.tensor.matmul(warm_ps[:, i % 2, 0:256], lhsT=junk_sb[:, 0:128],
                             rhs=junk_sb[:, 0:256], start=True, stop=True)
        for n in range(NE):
            fetch_feat(n)
            feat_sb = feat_tiles.pop(n)
            fetch_feat(n + 1)
            fetch_feat(n + 2)
            fetch_feat(n + 3)
            targ_sb = iopool.tile([E, BS], FP8, tag="targ")
            nc.sync.dma_start(out=targ_sb[:], in_=targd[n])
            if n < 2 * NG - 1:
                g, which = divmod(n + 1, 2)
                if which == 0:
                    nc.sync.dma_start(out=w1_sb[:, ts(g, GE), :],
                                      in_=w1d[:, ts(g, GE), :])
                else:
                    nc.sync.dma_start(out=w2_sb[:, ts(g, GE), :, :],
                                      in_=w2d[:, ts(g, GE), :, :])
            if n == 25:
                # first 3/4 of the stats is final; overlap the store
                nc.sync.dma_start(out=statsd[:, 0:96, :],
                                  in_=stats_sb[:, 0:96, :])

            hact = hpool.tile([128, NT, 2, BT], FP8)
            p01 = p23 = None
            for g in range(4):
                c, pr = divmod(g, 2)        # chunk, tile-pair of this group
                ph_t = php.tile([128, 2, BT], F32, name=f"ph{g}", tag="ph")
                for i in range(2):
                    nc.tensor.matmul(ph_t[:, i, :], lhsT=w1ap(n, c),
                                     rhs=feat_sb[:, ts(2 * pr + i, BT)],
                                     start=True, stop=True)
                nc.scalar.activation(hact[:, 2 * pr:2 * pr + 2, c, :],
                                     ph_t[:, :, :],
                                     mybir.ActivationFunctionType.Gelu,
                                     bias=b1f[:, c, n:n + 1], scale=ACT_SCALE)
                if pending is not None:
                    if g == 0:
                        p01p = ppp.tile([128, 2, BT], F32, name="p01p", tag="pp")
                        flush_tile(pending, 0, p01p)
                    elif g == 1:
                        flush_tile(pending, 1, p01p)
                    elif g == 2:
                        p23p = ppp.tile([128, 2, BT], F32, name="p23p", tag="pp")
                        flush_tile(pending, 2, p23p)
                    else:
                        flush_tile(pending, 3, p23p)
            pending = (n, hact, targ_sb)
        pf01 = ppp.tile([128, 2, BT], F32, name="pf01", tag="pp")
        for t in (0, 1):
            flush_tile(pending, t, pf01)
        pf23 = ppp.tile([128, 2, BT], F32, name="pf23", tag="pp")
        for t in (2, 3):
            flush_tile(pending, t, pf23)
        nc.sync.dma_start(out=statsd[:, 96:, :], in_=stats_sb[:, 96:, :])
    return nc


LAST_RESULTS = None


def kernel(features, target_features, W1, b1, W2, b2):
    global LAST_RESULTS
    f8 = ml_dtypes.float8_e4m3
    features = np.asarray(features)
    target_features = np.asarray(target_features)
    W1 = np.asarray(W1)
    b1 = np.asarray(b1)
    W2 = np.asarray(W2)
    b2 = np.asarray(b2)

    feat4 = (features * S_X).reshape(C, BS, NE, E).transpose(0, 2, 3, 1).astype(f8)
    targ4 = ((target_features - b2[None]) * S_W2).reshape(C, BS, NE, E) \
        .transpose(0, 2, 3, 1).astype(f8)
    w1h = (W1 * S_W1).transpose(1, 0, 2).astype(f8)                  # [E, NE, H]
    w2h = (W2 * S_W2).reshape(NE, 2, 128, E).transpose(2, 0, 1, 3).astype(f8)
    b1h = np.ascontiguousarray(
        b1.reshape(NE, 2, 128).transpose(2, 1, 0).astype(np.float32))  # [p, c, n]

    negi = (-np.eye(128)).astype(f8)
    head = np.ascontiguousarray(np.concatenate(
        [negi.view(np.uint8),
         b1h.view(np.uint8).reshape(128, 256),
         np.ascontiguousarray(w1h[:, 0, :]).view(np.uint8),
         np.ascontiguousarray(w2h[:, 0, :, :]).reshape(128, 256).view(np.uint8)],
        axis=1)).view(f8).reshape(128, 7, 128)

    nc = _build_nc()
    in_maps = [
        {"featT": np.ascontiguousarray(feat4[c]),
         "targT": np.ascontiguousarray(targ4[c]),
         "w1": w1h, "w2": w2h, "head": head}
        for c in range(C)
    ]
    res = run_bass_kernel_spmd(nc, in_maps, list(range(C)))
    LAST_RESULTS = res
    # stats[p, slot] = [n0, mean0, M2_0, n1, mean1, M2_1] over the two
    # 256-halves of 16*diff; sum(diff^2) = (M2 + n*mean^2) / 256.
    total = 0.0
    for r in res.results:
        st = r["stats"].astype(np.float64)
        total += (st[..., 2] + st[..., 0] * st[..., 1] ** 2
                  + st[..., 5] + st[..., 3] * st[..., 4] ** 2).sum()
    return np.array(total * TTR_SCALE / (B * NE * E), dtype=np.float32)


# revision 28
# speedup vs baseline: 1.0068x; 1.0050x over previous
"""Trainium2 Bass kernel for nn_BaselineDistiller: grouped-expert MLP + MSE loss.

reference:
    h    = einsum('bne,neh->bnh', features, W1) + b1
    g    = gelu(h)                      # exact (erf) gelu
    pred = einsum('bnh,nhe->bne', g, W2) + b2
    out  = mean((pred - target)^2)

Strategy (8 NeuronCores, data-parallel over batch; ~153-157us on HW):
  The kernel is ScalarE(gelu)-bound: 131072 gelu elems/partition/core at
  1 elem/cycle/1.2GHz = 109us floor (+ ~150ns/op overhead). Everything else
  is arranged to keep the ACT engine gap-free and shrink head/tail:
  * All inputs are fp8 e4m3 (feat*8, targ*16 with b2 folded, W1*8, W2*16)
    -> DMA-in halves to ~19MB/core (~57us) and matmuls run at fp8 rate.
  * mm1 (h.T = W1c.T @ feat.T per 128-row chunk) writes PSUM; ACT applies
    gelu with per-chunk bias b1 and scale 1/64 in 4 FD-1024 ops/expert
    (each op is single-chunk so the per-partition bias is uniform), writing
    fp8 hact laid out [128, 4tiles, 2ktiles, 512] (k-tiles contiguous) so
    mm2 runs as ONE fp8 DoubleRow matmul per tile (K=256 at 1 col/cycle —
    2x FLOPs; measured same 216ns as a K=128 matmul).
  * pred tiles accumulate [W2-DR, -I @ (16*(targ-b2))] so PSUM ends holding
    16*diff; one DVE bn_stats per tile (the only DVE reduction needing a
    single PSUM read) yields per-256-group {n, mean, M2}; the host
    reconstructs sum(diff^2) = sum M2 + n*mean^2, dividing out the 16^2.
  * PSUM: ph pool (2 bufs x 2 banks) is a pure mm1->ACT ping-pong; pp pool
    (2 bufs x 2 banks) holds pred pairs. Each expert emits 4 symmetric
    sub-blocks [mm1 pair, ACT, mm2+bn of the PREVIOUS expert's tile k], so
    every cross-engine chain has ~0.5us of slack at the ACT cadence and the
    in-order PE never starves the ACT queue (measured <2us of ACT gaps
    after the DMA ramp; PE ~77%, DVE ~62% of the cadence).
  * A dummy gelu at t=0 pulls the 1.3us ACT table load into the DMA ramp;
    20 short junk matmuls bridge the ramp so the first real mm1 runs at
    full PE p-state; feat DMAs are prefetched 3 experts ahead and the first
    weight group is issued with the head so expert 1-2 mm1s never wait;
    stats ship in a 96/32 split so the tail only waits on the last expert.
    Typical HW time ~152-156us at nominal clocks (the chip DVFS-throttles
    some runs ~20%; the schedule stays gap-free either way).
"""

import contextlib
import ctypes
import json
import sys
import types

import ml_dtypes
import numpy as np

import concourse.bass as bass
import concourse.mybir as mybir
import concourse.tile as tile
from concourse import bass_utils
from concourse.bass import ts
from concourse.bass_utils import run_bass_kernel_spmd

B, NE, E, H = 16384, 32, 128, 256
C = 8              # cores
BS = B // C        # batch rows per core
BT = 512           # batch columns per matmul tile
NT = BS // BT      # 4
FP8 = mybir.dt.float8e4
F32 = mybir.dt.float32
DR = mybir.MatmulPerfMode.DoubleRow

S_X = 8.0          # feature scale into fp8
S_W1 = 8.0
S_W2 = 16.0        # also the target scale (so pred/targ match in PSUM)
ACT_SCALE = 1.0 / (S_X * S_W1)
TTR_SCALE = 1.0 / (S_W2 * S_W2)

# ---------------------------------------------------------------------------
# Environment shims (idempotent):
#  1. antenv.axon_hooks — the image's antenv lacks it; provide the NTFF
#     profile hook via ctypes so trace=True works when a caller requests it.
#  2. upload_artifacts — no bucket access in this container; keep local.
#  3. This walrus build rejects instructions with >1 sync-wait; split the
#     extra waits onto NoOps at BIR-serialization time.
# ---------------------------------------------------------------------------
_AXON_SO = "/opt/axon/libaxon_pjrt.so"


def _make_ntff_hook(so_path):
    try:
        lib = ctypes.CDLL(so_path)
    except OSError:
        return None
    if not hasattr(lib, "axon_start_nrt_profile"):
        return None
    lib.axon_start_nrt_profile.argtypes = [ctypes.POINTER(ctypes.c_int64), ctypes.c_size_t]
    lib.axon_start_nrt_profile.restype = ctypes.c_int64
    lib.axon_stop_nrt_profile.argtypes = [ctypes.c_char_p]
    lib.axon_stop_nrt_profile.restype = ctypes.c_int64

    @contextlib.contextmanager
    def _hook(output_dir, device_ids):
        import jax

        jax.devices()
        if device_ids:
            ids = (ctypes.c_int64 * len(device_ids))(*device_ids)
            rc = lib.axon_start_nrt_profile(ids, len(device_ids))
        else:
            rc = lib.axon_start_nrt_profile(None, 0)
        if rc != 0:
            raise RuntimeError(f"axon_start_nrt_profile rc={rc}")
        try:
            yield
        finally:
            n = lib.axon_stop_nrt_profile(str(output_dir).encode())
            print(f"profile: {n} file(s) written to {output_dir}", file=sys.stderr)

    return _hook


if "antenv.axon_hooks" not in sys.modules:
    _mod = types.ModuleType("antenv.axon_hooks")
    _the_hook = _make_ntff_hook(_AXON_SO)
    _mod.get_axon_ntff_profile_hook = lambda: _the_hook
    sys.modules["antenv.axon_hooks"] = _mod

bass_utils.upload_artifacts = lambda tmpdir: str(tmpdir)

_MAXW = 1
if not getattr(bass.Bass, "_wait_split_installed", False):
    _orig_to_json_bytes = bass.Bass.to_json_bytes

    def _split_sync_waits(self, *a, **kw):
        bir = json.loads(_orig_to_json_bytes(self, *a, **kw))
        for fn in bir.get("functions", []):
            for blk in fn.get("blocks", []):
                new_insts = []
                for inst in blk.get("instructions", []):
                    si = inst.get("sync_info") or {}
                    waits = si.get("on_wait") or []
                    if len(waits) > _MAXW:
                        extra, keep = waits[:-_MAXW], waits[-_MAXW:]
                        for k in range(0, len(extra), _MAXW):
                            new_insts.append({
                                "debug": inst.get("debug", 0),
                                "engine": inst["engine"],
                                "ins": [], "outs": [],
                                "name": f"{inst['name']}_wsplit{k}",
                                "opcode": "NoOp",
                                "sync_info": {"on_update": [],
                                              "on_wait": extra[k:k + _MAXW]},
                            })
                        si["on_wait"] = keep
                    new_insts.append(inst)
                blk["instructions"] = new_insts
        return json.dumps(bir).encode()

    bass.Bass.to_json_bytes = _split_sync_waits
    bass.Bass._wait_split_installed = True


# ---------------------------------------------------------------------------
# Device kernel
# ---------------------------------------------------------------------------
def _build_nc():
    nc = bass.Bass("TRN2", target_bir_lowering=False, debug=False)
    featd = nc.declare_dram_parameter("featT", [NE, E, BS], FP8, isOutput=False)
    targd = nc.declare_dram_parameter("targT", [NE, E, BS], FP8, isOutput=False)
    w1d = nc.declare_dram_parameter("w1", [E, NE, H], FP8, isOutput=False)
    w2d = nc.declare_dram_parameter("w2", [128, NE, 2, E], FP8, isOutput=False)
    # head = [negI | b1(f32-as-bytes) | W1[e0] | W2[e0]] so one DMA unblocks
    # expert 0 entirely.
    headd = nc.declare_dram_parameter("head", [128, 7, 128], FP8, isOutput=False)
    statsd = nc.declare_dram_parameter("stats", [128, 4 * NE, 6], F32, isOutput=True)

    GE = 8                     # experts per weight-DMA group
    NG = NE // GE

    with tile.TileContext(nc) as tc, contextlib.ExitStack() as ctx:
        wpool = ctx.enter_context(tc.tile_pool(name="weights", bufs=1))
        iopool = ctx.enter_context(tc.tile_pool(name="io", bufs=4))
        hpool = ctx.enter_context(tc.tile_pool(name="hact", bufs=3))
        php = ctx.enter_context(tc.tile_pool(name="ph", bufs=2, space="PSUM"))
        ppp = ctx.enter_context(tc.tile_pool(name="pp", bufs=2, space="PSUM"))

        head_sb = wpool.tile([128, 7, 128], FP8)
        negi_sb = head_sb[:, 0, :]
        b1f = head_sb[:, 1:3, :].bitcast(F32)        # [128, 2, 32] (p, c, n)
        w1e0 = head_sb[:, 3:5, :]                    # [128, 2, 128] (p, c, m)
        w2e0 = head_sb[:, 5:7, :]                    # [128, 2, 128] DR lhsT
        w1_sb = wpool.tile([E, NE, H], FP8)          # [128, 32, 256]
        w2_sb = wpool.tile([128, NE, 2, E], FP8)
        stats_sb = wpool.tile([128, 4 * NE, 6], F32)
        warm_sb = wpool.tile([128, 1], F32)

        # Load the gelu table during the DMA ramp instead of right before the
        # first real activation.
        nc.gpsimd.memset(warm_sb[:], 0.0)
        nc.scalar.activation(warm_sb[:], warm_sb[:],
                             mybir.ActivationFunctionType.Gelu)

        def w1ap(n, c):
            return w1e0[:, c, :] if n == 0 else w1_sb[:, n, ts(c, 128)]

        def w2ap(n):
            return w2e0 if n == 0 else w2_sb[:, n, :, :]

        # flush tile t of the previous expert: mm2 (DoubleRow) + negI + bn
        def flush_tile(pend, t, pair_tile):
            n, hact, targ_sb = pend
            j = t % 2
            nc.tensor.matmul(pair_tile[:, j, :], lhsT=w2ap(n),
                             rhs=hact[:, t, :, :],
                             start=True, stop=False, perf_mode=DR)
            nc.tensor.matmul(pair_tile[:, j, :], lhsT=negi_sb,
                             rhs=targ_sb[:, ts(t, BT)],
                             start=False, stop=True)
            nc.vector.bn_stats(out=stats_sb[:, 4 * n + t, :],
                               in_=pair_tile[:, j, :])

        pending = None
        feat_tiles = {}

        def fetch_feat(n):
            if n < NE and n not in feat_tiles:
                f = iopool.tile([E, BS], FP8, tag="feat", name="feat_sb")
                nc.sync.dma_start(out=f[:], in_=featd[n])
                feat_tiles[n] = f

        f0 = iopool.tile([E, BS], FP8, tag="feat", name="feat_sb")
        nc.sync.dma_start(out=f0[:, 0:BS // 2], in_=featd[0][:, 0:BS // 2])
        nc.sync.dma_start(out=head_sb[:], in_=headd[:])
        nc.sync.dma_start(out=f0[:, BS // 2:], in_=featd[0][:, BS // 2:])
        feat_tiles[0] = f0
        nc.sync.dma_start(out=w1_sb[:, ts(0, GE), :], in_=w1d[:, ts(0, GE), :])
        # PE warm-up: keep the tensor engine busy through the DMA ramp so the
        # first real matmuls run at full p-state instead of cold-start speed.
        junk_sb = wpool.tile([128, 512], FP8)
        nc.gpsimd.memset(junk_sb[:], 0.0)
        warm_ps = php.tile([128, 2, BT], F32, name="warmps", tag="ph")
        for i in range(20):
            nc.tensor.matmul(warm_ps[:, i % 2, 0:256], lhsT=junk_sb[:, 0:128],
                             rhs=junk_sb[:, 0:256], start=True, stop=True)
        for n in range(NE):
            fetch_feat(n)
            feat_sb = feat_tiles.pop(n)
            fetch_feat(n + 1)
            fetch_feat(n + 2)
            fetch_feat(n + 3)
            targ_sb = iopool.tile([E, BS], FP8, tag="targ")
            nc.sync.dma_start(out=targ_sb[:], in_=targd[n])
            if n < 2 * NG - 1:
                g, which = divmod(n + 1, 2)
                if which == 0:
                    nc.sync.dma_start(out=w1_sb[:, ts(g, GE), :],
                                      in_=w1d[:, ts(g, GE), :])
                else:
                    nc.sync.dma_start(out=w2_sb[:, ts(g, GE), :, :],
                                      in_=w2d[:, ts(g, GE), :, :])
            if n == 25:
                # first 3/4 of the stats is final; overlap the store
                nc.sync.dma_start(out=statsd[:, 0:96, :],
                                  in_=stats_sb[:, 0:96, :])

            hact = hpool.tile([128, NT, 2, BT], FP8)
            p01 = p23 = None
            for g in range(4):
                c, pr = divmod(g, 2)        # chunk, tile-pair of this group
                ph_t = php.tile([128, 2, BT], F32, name=f"ph{g}", tag="ph")
                for i in range(2):
                    nc.tensor.matmul(ph_t[:, i, :], lhsT=w1ap(n, c),
                                     rhs=feat_sb[:, ts(2 * pr + i, BT)],
                                     start=True, stop=True)
                nc.scalar.activation(hact[:, 2 * pr:2 * pr + 2, c, :],
                                     ph_t[:, :, :],
                                     mybir.ActivationFunctionType.Gelu,
                                     bias=b1f[:, c, n:n + 1], scale=ACT_SCALE)
                if pending is not None:
                    if g == 0:
                        p01p = ppp.tile([128, 2, BT], F32, name="p01p", tag="pp")
                        flush_tile(pending, 0, p01p)
                    elif g == 1:
                        flush_tile(pending, 1, p01p)
                    elif g == 2:
                        p23p = ppp.tile([128, 2, BT], F32, name="p23p", tag="pp")
                        flush_tile(pending, 2, p23p)
                    else:
                        flush_tile(pending, 3, p23p)
            pending = (n, hact, targ_sb)
        pf01 = ppp.tile([128, 2, BT], F32, name="pf01", tag="pp")
        for t in (0, 1):
            flush_tile(pending, t, pf01)
        pf23 = ppp.tile([128, 2, BT], F32, name="pf23", tag="pp")
        for t in (2, 3):
            flush_tile(pending, t, pf23)
        nc.sync.dma_start(out=statsd[:, 96:, :], in_=stats_sb[:, 96:, :])
    return nc


LAST_RESULTS = None


def kernel(features, target_features, W1, b1, W2, b2):
    global LAST_RESULTS
    f8 = ml_dtypes.float8_e4m3
    features = np.asarray(features)
    target_features = np.asarray(target_features)
    W1 = np.asarray(W1)
    b1 = np.asarray(b1)
    W2 = np.asarray(W2)
    b2 = np.asarray(b2)

    feat4 = (features * S_X).reshape(C, BS, NE, E).transpose(0, 2, 3, 1).astype(f8)
    targ4 = ((target_features - b2[None]) * S_W2).reshape(C, BS, NE, E) \
        .transpose(0, 2, 3, 1).astype(f8)
    w1h = (W1 * S_W1).transpose(1, 0, 2).astype(f8)                  # [E, NE, H]
    w2h = (W2 * S_W2).reshape(NE, 2, 128, E).transpose(2, 0, 1, 3).astype(f8)
    b1h = np.ascontiguousarray(
        b1.reshape(NE, 2, 128).transpose(2, 1, 0).astype(np.float32))  # [p, c, n]

    negi = (-np.eye(128)).astype(f8)
    head = np.ascontiguousarray(np.concatenate(
        [negi.view(np.uint8),
         b1h.view(np.uint8).reshape(128, 256),
         np.ascontiguousarray(w1h[:, 0, :]).view(np.uint8),
         np.ascontiguousarray(w2h[:, 0, :, :]).reshape(128, 256).view(np.uint8)],
        axis=1)).view(f8).reshape(128, 7, 128)

    nc = _build_nc()
    in_maps = [
        {"featT": np.ascontiguousarray(feat4[c]),
         "targT": np.ascontiguousarray(targ4[c]),
         "w1": w1h, "w2": w2h, "head": head}
        for c in range(C)
    ]
    res = run_bass_kernel_spmd(nc, in_maps, list(range(C)))
    LAST_RESULTS = res
    # stats[p, slot] = [n0, mean0, M2_0, n1, mean1, M2_1] over the two
    # 256-halves of 16*diff; sum(diff^2) = (M2 + n*mean^2) / 256.
    total = 0.0
    for r in res.results:
        st = r["stats"].astype(np.float64)
        total += (st[..., 2] + st[..., 0] * st[..., 1] ** 2
                  + st[..., 5] + st[..., 3] * st[..., 4] ** 2).sum()
    return np.array(total * TTR_SCALE / (B * NE * E), dtype=np.float32)


# revision 29
# speedup vs baseline: 1.0069x; 1.0000x over previous
"""Trainium2 Bass kernel for nn_BaselineDistiller: grouped-expert MLP + MSE loss.

reference:
    h    = einsum('bne,neh->bnh', features, W1) + b1
    g    = gelu(h)                      # exact (erf) gelu
    pred = einsum('bnh,nhe->bne', g, W2) + b2
    out  = mean((pred - target)^2)

Strategy (8 NeuronCores, data-parallel over batch; ~153-157us on HW):
  The kernel is ScalarE(gelu)-bound: 131072 gelu elems/partition/core at
  1 elem/cycle/1.2GHz = 109us floor (+ ~150ns/op overhead). Everything else
  is arranged to keep the ACT engine gap-free and shrink head/tail:
  * All inputs are fp8 e4m3 (feat*8, targ*16 with b2 folded, W1*8, W2*16)
    -> DMA-in halves to ~19MB/core (~57us) and matmuls run at fp8 rate.
  * mm1 (h.T = W1c.T @ feat.T per 128-row chunk) writes PSUM; ACT applies
    gelu with per-chunk bias b1 and scale 1/64 in 4 FD-1024 ops/expert
    (each op is single-chunk so the per-partition bias is uniform), writing
    fp8 hact laid out [128, 4tiles, 2ktiles, 512] (k-tiles contiguous) so
    mm2 runs as ONE fp8 DoubleRow matmul per tile (K=256 at 1 col/cycle —
    2x FLOPs; measured same 216ns as a K=128 matmul).
  * pred tiles accumulate [W2-DR, -I @ (16*(targ-b2))] so PSUM ends holding
    16*diff; one DVE bn_stats per tile (the only DVE reduction needing a
    single PSUM read) yields per-256-group {n, mean, M2}; the host
    reconstructs sum(diff^2) = sum M2 + n*mean^2, dividing out the 16^2.
  * PSUM: ph pool (2 bufs x 2 banks) is a pure mm1->ACT ping-pong; pp pool
    (2 bufs x 2 banks) holds pred pairs. Each expert emits 4 symmetric
    sub-blocks [mm1 pair, ACT, mm2+bn of the PREVIOUS expert's tile k], so
    every cross-engine chain has ~0.5us of slack at the ACT cadence and the
    in-order PE never starves the ACT queue (measured <2us of ACT gaps
    after the DMA ramp; PE ~77%, DVE ~62% of the cadence).
  * A dummy gelu at t=0 pulls the 1.3us ACT table load into the DMA ramp;
    20 short junk matmuls bridge the ramp so the first real mm1 runs at
    full PE p-state; feat DMAs are prefetched 3 experts ahead and the first
    weight group is issued with the head so expert 1-2 mm1s never wait;
    stats ship in a 96/32 split so the tail only waits on the last expert.
    Typical HW time ~152-156us at nominal clocks (the chip DVFS-throttles
    some runs ~20%; the schedule stays gap-free either way).
"""

import contextlib
import ctypes
import json
import sys
import types

import ml_dtypes
import numpy as np

import concourse.bass as bass
import concourse.mybir as mybir
import concourse.tile as tile
from concourse import bass_utils
from concourse.bass import ts
from concourse.bass_utils import run_bass_kernel_spmd

B, NE, E, H = 16384, 32, 128, 256
C = 8              # cores
BS = B // C        # batch rows per core
BT = 512           # batch columns per matmul tile
NT = BS // BT      # 4
FP8 = mybir.dt.float8e4
F32 = mybir.dt.float32
DR = mybir.MatmulPerfMode.DoubleRow

S_X = 8.0          # feature scale into fp8
S_W1 = 8.0
S_W2 = 16.0        # also the target scale (so pred/targ match in PSUM)
ACT_SCALE = 1.0 / (S_X * S_W1)
TTR_SCALE = 1.0 / (S_W2 * S_W2)

# ---------------------------------------------------------------------------
# Environment shims (idempotent):
#  1. antenv.axon_hooks — the image's antenv lacks it; provide the NTFF
#     profile hook via ctypes so trace=True works when a caller requests it.
#  2. upload_artifacts — no bucket access in this container; keep local.
#  3. This walrus build rejects instructions with >1 sync-wait; split the
#     extra waits onto NoOps at BIR-serialization time.
# ---------------------------------------------------------------------------
_AXON_SO = "/opt/axon/libaxon_pjrt.so"


def _make_ntff_hook(so_path):
    try:
        lib = ctypes.CDLL(so_path)
    except OSError:
        return None
    if not hasattr(lib, "axon_start_nrt_profile"):
        return None
    lib.axon_start_nrt_profile.argtypes = [ctypes.POINTER(ctypes.c_int64), ctypes.c_size_t]
    lib.axon_start_nrt_profile.restype = ctypes.c_int64
    lib.axon_stop_nrt_profile.argtypes = [ctypes.c_char_p]
    lib.axon_stop_nrt_profile.restype = ctypes.c_int64

    @contextlib.contextmanager
    def _hook(output_dir, device_ids):
        import jax

        jax.devices()
        if device_ids:
            ids = (ctypes.c_int64 * len(device_ids))(*device_ids)
            rc = lib.axon_start_nrt_profile(ids, len(device_ids))
        else:
            rc = lib.axon_start_nrt_profile(None, 0)
        if rc != 0:
            raise RuntimeError(f"axon_start_nrt_profile rc={rc}")
        try:
            yield
        finally:
            n = lib.axon_stop_nrt_profile(str(output_dir).encode())
            print(f"profile: {n} file(s) written to {output_dir}", file=sys.stderr)

    return _hook


if "antenv.axon_hooks" not in sys.modules:
    _mod = types.ModuleType("antenv.axon_hooks")
    _the_hook = _make_ntff_hook(_AXON_SO)
    _mod.get_axon_ntff_profile_hook = lambda: _the_hook
    sys.modules["antenv.axon_hooks"] = _mod

bass_utils.upload_artifacts = lambda tmpdir: str(tmpdir)

_MAXW = 1
if not getattr(bass.Bass, "_wait_split_installed", False):
    _orig_to_json_bytes = bass.Bass.to_json_bytes

    def _split_sync_waits(self, *a, **kw):
        bir = json.loads(_orig_to_json_bytes(self, *a, **kw))
        for fn in bir.get("functions", []):
            for blk in fn.get("blocks", []):
                new_insts = []
                for inst in blk.get("instructions", []):
                    si = inst.get("sync_info") or {}
                    waits = si.get("on_wait") or []
                    if len(waits) > _MAXW:
                        extra, keep = waits[:-_MAXW], waits[-_MAXW:]
                        for k in range(0, len(extra), _MAXW):
                            new_insts.append({
                                "debug": inst.get("debug", 0),
                                "engine": inst["engine"],
                                "ins": [], "outs": [],
                                "name": f"{inst['name']}_wsplit{k}",
                                "opcode": "NoOp",
                                "sync_info": {"on_update": [],
                                              "on_wait": extra[k:k + _MAXW]},
                            })
                        si["on_wait"] = keep
                    new_insts.append(inst)
                blk["instructions"] = new_insts
        return json.dumps(bir).encode()

    bass.Bass.to_json_bytes = _split_sync_waits
    bass.Bass._wait_split_installed = True


# ---------------------------------------------------------------------------
# Device kernel
# ---------------------------------------------------------------------------
def _build_nc():
    nc = bass.Bass("TRN2", target_bir_lowering=False, debug=False)
    featd = nc.declare_dram_parameter("featT", [NE, E, BS], FP8, isOutput=False)
    targd = nc.declare_dram_parameter("targT", [NE, E, BS], FP8, isOutput=False)
    w1d = nc.declare_dram_parameter("w1", [E, NE, H], FP8, isOutput=False)
    w2d = nc.declare_dram_parameter("w2", [128, NE, 2, E], FP8, isOutput=False)
    # head = [negI | b1(f32-as-bytes) | W1[e0] | W2[e0]] so one DMA unblocks
    # expert 0 entirely.
    headd = nc.declare_dram_parameter("head", [128, 7, 128], FP8, isOutput=False)
    statsd = nc.declare_dram_parameter("stats", [128, 4 * NE, 6], F32, isOutput=True)

    GE = 8                     # experts per weight-DMA group
    NG = NE // GE

    with tile.TileContext(nc) as tc, contextlib.ExitStack() as ctx:
        wpool = ctx.enter_context(tc.tile_pool(name="weights", bufs=1))
        iopool = ctx.enter_context(tc.tile_pool(name="io", bufs=4))
        hpool = ctx.enter_context(tc.tile_pool(name="hact", bufs=3))
        php = ctx.enter_context(tc.tile_pool(name="ph", bufs=2, space="PSUM"))
        ppp = ctx.enter_context(tc.tile_pool(name="pp", bufs=2, space="PSUM"))

        head_sb = wpool.tile([128, 7, 128], FP8)
        negi_sb = head_sb[:, 0, :]
        b1f = head_sb[:, 1:3, :].bitcast(F32)        # [128, 2, 32] (p, c, n)
        w1e0 = head_sb[:, 3:5, :]                    # [128, 2, 128] (p, c, m)
        w2e0 = head_sb[:, 5:7, :]                    # [128, 2, 128] DR lhsT
        w1_sb = wpool.tile([E, NE, H], FP8)          # [128, 32, 256]
        w2_sb = wpool.tile([128, NE, 2, E], FP8)
        stats_sb = wpool.tile([128, 4 * NE, 6], F32)
        warm_sb = wpool.tile([128, 1], F32)

        # head rides the Activation engine's DGE queue so its descriptor
        # generation runs in parallel with feat0's on SP; the gelu table
        # then loads during the DMA ramp.
        nc.scalar.dma_start(out=head_sb[:], in_=headd[:])
        nc.gpsimd.memset(warm_sb[:], 0.0)
        nc.scalar.activation(warm_sb[:], warm_sb[:],
                             mybir.ActivationFunctionType.Gelu)

        def w1ap(n, c):
            return w1e0[:, c, :] if n == 0 else w1_sb[:, n, ts(c, 128)]

        def w2ap(n):
            return w2e0 if n == 0 else w2_sb[:, n, :, :]

        # flush tile t of the previous expert: mm2 (DoubleRow) + negI + bn
        def flush_tile(pend, t, pair_tile):
            n, hact, targ_sb = pend
            j = t % 2
            nc.tensor.matmul(pair_tile[:, j, :], lhsT=w2ap(n),
                             rhs=hact[:, t, :, :],
                             start=True, stop=False, perf_mode=DR)
            nc.tensor.matmul(pair_tile[:, j, :], lhsT=negi_sb,
                             rhs=targ_sb[:, ts(t, BT)],
                             start=False, stop=True)
            nc.vector.bn_stats(out=stats_sb[:, 4 * n + t, :],
                               in_=pair_tile[:, j, :])

        pending = None
        feat_tiles = {}

        def fetch_feat(n):
            if n < NE and n not in feat_tiles:
                f = iopool.tile([E, BS], FP8, tag="feat", name="feat_sb")
                nc.sync.dma_start(out=f[:], in_=featd[n])
                feat_tiles[n] = f

        fetch_feat(0)
        nc.sync.dma_start(out=w1_sb[:, ts(0, GE), :], in_=w1d[:, ts(0, GE), :])
        # PE warm-up: keep the tensor engine busy through the DMA ramp so the
        # first real matmuls run at full p-state instead of cold-start speed.
        junk_sb = wpool.tile([128, 512], FP8)
        nc.gpsimd.memset(junk_sb[:], 0.0)
        warm_ps = php.tile([128, 2, BT], F32, name="warmps", tag="ph")
        for i in range(20):
            nc.tensor.matmul(warm_ps[:, i % 2, 0:256], lhsT=junk_sb[:, 0:128],
                             rhs=junk_sb[:, 0:256], start=True, stop=True)
        for n in range(NE):
            fetch_feat(n)
            feat_sb = feat_tiles.pop(n)
            fetch_feat(n + 1)
            fetch_feat(n + 2)
            fetch_feat(n + 3)
            targ_sb = iopool.tile([E, BS], FP8, tag="targ")
            nc.sync.dma_start(out=targ_sb[:], in_=targd[n])
            if n < 2 * NG - 1:
                g, which = divmod(n + 1, 2)
                if which == 0:
                    nc.sync.dma_start(out=w1_sb[:, ts(g, GE), :],
                                      in_=w1d[:, ts(g, GE), :])
                else:
                    nc.sync.dma_start(out=w2_sb[:, ts(g, GE), :, :],
                                      in_=w2d[:, ts(g, GE), :, :])
            if n == 25:
                # first 3/4 of the stats is final; overlap the store
                nc.sync.dma_start(out=statsd[:, 0:96, :],
                                  in_=stats_sb[:, 0:96, :])

            hact = hpool.tile([128, NT, 2, BT], FP8)
            p01 = p23 = None
            for g in range(4):
                c, pr = divmod(g, 2)        # chunk, tile-pair of this group
                ph_t = php.tile([128, 2, BT], F32, name=f"ph{g}", tag="ph")
                for i in range(2):
                    nc.tensor.matmul(ph_t[:, i, :], lhsT=w1ap(n, c),
                                     rhs=feat_sb[:, ts(2 * pr + i, BT)],
                                     start=True, stop=True)
                nc.scalar.activation(hact[:, 2 * pr:2 * pr + 2, c, :],
                                     ph_t[:, :, :],
                                     mybir.ActivationFunctionType.Gelu,
                                     bias=b1f[:, c, n:n + 1], scale=ACT_SCALE)
                if pending is not None:
                    if g == 0:
                        p01p = ppp.tile([128, 2, BT], F32, name="p01p", tag="pp")
                        flush_tile(pending, 0, p01p)
                    elif g == 1:
                        flush_tile(pending, 1, p01p)
                    elif g == 2:
                        p23p = ppp.tile([128, 2, BT], F32, name="p23p", tag="pp")
                        flush_tile(pending, 2, p23p)
                    else:
                        flush_tile(pending, 3, p23p)
            pending = (n, hact, targ_sb)
        pf01 = ppp.tile([128, 2, BT], F32, name="pf01", tag="pp")
        for t in (0, 1):
            flush_tile(pending, t, pf01)
        pf23 = ppp.tile([128, 2, BT], F32, name="pf23", tag="pp")
        for t in (2, 3):
            flush_tile(pending, t, pf23)
        nc.sync.dma_start(out=statsd[:, 96:, :], in_=stats_sb[:, 96:, :])
    return nc


LAST_RESULTS = None


def kernel(features, target_features, W1, b1, W2, b2):
    global LAST_RESULTS
    f8 = ml_dtypes.float8_e4m3
    features = np.asarray(features)
    target_features = np.asarray(target_features)
    W1 = np.asarray(W1)
    b1 = np.asarray(b1)
    W2 = np.asarray(W2)
    b2 = np.asarray(b2)

    feat4 = (features * S_X).reshape(C, BS, NE, E).transpose(0, 2, 3, 1).astype(f8)
    targ4 = ((target_features - b2[None]) * S_W2).reshape(C, BS, NE, E) \
        .transpose(0, 2, 3, 1).astype(f8)
    w1h = (W1 * S_W1).transpose(1, 0, 2).astype(f8)                  # [E, NE, H]
    w2h = (W2 * S_W2).reshape(NE, 2, 128, E).transpose(2, 0, 1, 3).astype(f8)
    b1h = np.ascontiguousarray(
        b1.reshape(NE, 2, 128).transpose(2, 1, 0).astype(np.float32))  # [p, c, n]

    negi = (-np.eye(128)).astype(f8)
    head = np.ascontiguousarray(np.concatenate(
        [negi.view(np.uint8),
         b1h.view(np.uint8).reshape(128, 256),
         np.ascontiguousarray(w1h[:, 0, :]).view(np.uint8),
         np.ascontiguousarray(w2h[:, 0, :, :]).reshape(128, 256).view(np.uint8)],
        axis=1)).view(f8).reshape(128, 7, 128)

    nc = _build_nc()
    in_maps = [
        {"featT": np.ascontiguousarray(feat4[c]),
         "targT": np.ascontiguousarray(targ4[c]),
         "w1": w1h, "w2": w2h, "head": head}
        for c in range(C)
    ]
    res = run_bass_kernel_spmd(nc, in_maps, list(range(C)))
    LAST_RESULTS = res
    # stats[p, slot] = [n0, mean0, M2_0, n1, mean1, M2_1] over the two
    # 256-halves of 16*diff; sum(diff^2) = (M2 + n*mean^2) / 256.
    total = 0.0
    for r in res.results:
        st = r["stats"].astype(np.float64)
        total += (st[..., 2] + st[..., 0] * st[..., 1] ** 2
                  + st[..., 5] + st[..., 3] * st[..., 4] ** 2).sum()
    return np.array(total * TTR_SCALE / (B * NE * E), dtype=np.float32)


# revision 30
# speedup vs baseline: 1.0104x; 1.0035x over previous
"""Trainium2 Bass kernel for nn_BaselineDistiller: grouped-expert MLP + MSE loss.

reference:
    h    = einsum('bne,neh->bnh', features, W1) + b1
    g    = gelu(h)                      # exact (erf) gelu
    pred = einsum('bnh,nhe->bne', g, W2) + b2
    out  = mean((pred - target)^2)

Strategy (8 NeuronCores, data-parallel over batch; ~153-157us on HW):
  The kernel is ScalarE(gelu)-bound: 131072 gelu elems/partition/core at
  1 elem/cycle/1.2GHz = 109us floor (+ ~150ns/op overhead). Everything else
  is arranged to keep the ACT engine gap-free and shrink head/tail:
  * All inputs are fp8 e4m3 (feat*8, targ*16 with b2 folded, W1*8, W2*16)
    -> DMA-in halves to ~19MB/core (~57us) and matmuls run at fp8 rate.
  * mm1 (h.T = W1c.T @ feat.T per 128-row chunk) writes PSUM; ACT applies
    gelu with per-chunk bias b1 and scale 1/64 in 4 FD-1024 ops/expert
    (each op is single-chunk so the per-partition bias is uniform), writing
    fp8 hact laid out [128, 4tiles, 2ktiles, 512] (k-tiles contiguous) so
    mm2 runs as ONE fp8 DoubleRow matmul per tile (K=256 at 1 col/cycle —
    2x FLOPs; measured same 216ns as a K=128 matmul).
  * pred tiles accumulate [W2-DR, -I @ (16*(targ-b2))] so PSUM ends holding
    16*diff; one DVE bn_stats per tile (the only DVE reduction needing a
    single PSUM read) yields per-256-group {n, mean, M2}; the host
    reconstructs sum(diff^2) = sum M2 + n*mean^2, dividing out the 16^2.
  * PSUM: ph pool (2 bufs x 2 banks) is a pure mm1->ACT ping-pong; pp pool
    (2 bufs x 2 banks) holds pred pairs. Each expert emits 4 symmetric
    sub-blocks [mm1 pair, ACT, mm2+bn of the PREVIOUS expert's tile k], so
    every cross-engine chain has ~0.5us of slack at the ACT cadence and the
    in-order PE never starves the ACT queue (measured <2us of ACT gaps
    after the DMA ramp; PE ~77%, DVE ~62% of the cadence).
  * A dummy gelu at t=0 pulls the 1.3us ACT table load into the DMA ramp;
    20 short junk matmuls bridge the ramp so the first real mm1 runs at
    full PE p-state; feat DMAs are prefetched 3 experts ahead and the first
    weight group is issued with the head so expert 1-2 mm1s never wait;
    stats ship in a 96/32 split so the tail only waits on the last expert.
    Typical HW time ~152-156us at nominal clocks (the chip DVFS-throttles
    some runs ~20%; the schedule stays gap-free either way).
"""

import contextlib
import ctypes
import json
import sys
import types

import ml_dtypes
import numpy as np

import concourse.bass as bass
import concourse.mybir as mybir
import concourse.tile as tile
from concourse import bass_utils
from concourse.bass import ts
from concourse.bass_utils import run_bass_kernel_spmd

B, NE, E, H = 16384, 32, 128, 256
C = 8              # cores
BS = B // C        # batch rows per core
BT = 512           # batch columns per matmul tile
NT = BS // BT      # 4
FP8 = mybir.dt.float8e4
F32 = mybir.dt.float32
DR = mybir.MatmulPerfMode.DoubleRow

S_X = 8.0          # feature scale into fp8
S_W1 = 8.0
S_W2 = 16.0        # also the target scale (so pred/targ match in PSUM)
ACT_SCALE = 1.0 / (S_X * S_W1)
TTR_SCALE = 1.0 / (S_W2 * S_W2)

# ---------------------------------------------------------------------------
# Environment shims (idempotent):
#  1. antenv.axon_hooks — the image's antenv lacks it; provide the NTFF
#     profile hook via ctypes so trace=True works when a caller requests it.
#  2. upload_artifacts — no bucket access in this container; keep local.
#  3. This walrus build rejects instructions with >1 sync-wait; split the
#     extra waits onto NoOps at BIR-serialization time.
# ---------------------------------------------------------------------------
_AXON_SO = "/opt/axon/libaxon_pjrt.so"


def _make_ntff_hook(so_path):
    try:
        lib = ctypes.CDLL(so_path)
    except OSError:
        return None
    if not hasattr(lib, "axon_start_nrt_profile"):
        return None
    lib.axon_start_nrt_profile.argtypes = [ctypes.POINTER(ctypes.c_int64), ctypes.c_size_t]
    lib.axon_start_nrt_profile.restype = ctypes.c_int64
    lib.axon_stop_nrt_profile.argtypes = [ctypes.c_char_p]
    lib.axon_stop_nrt_profile.restype = ctypes.c_int64

    @contextlib.contextmanager
    def _hook(output_dir, device_ids):
        import jax

        jax.devices()
        if device_ids:
            ids = (ctypes.c_int64 * len(device_ids))(*device_ids)
            rc = lib.axon_start_nrt_profile(ids, len(device_ids))
        else:
            rc = lib.axon_start_nrt_profile(None, 0)
        if rc != 0:
            raise RuntimeError(f"axon_start_nrt_profile rc={rc}")
        try:
            yield
        finally:
            n = lib.axon_stop_nrt_profile(str(output_dir).encode())
            print(f"profile: {n} file(s) written to {output_dir}", file=sys.stderr)

    return _hook


if "antenv.axon_hooks" not in sys.modules:
    _mod = types.ModuleType("antenv.axon_hooks")
    _the_hook = _make_ntff_hook(_AXON_SO)
    _mod.get_axon_ntff_profile_hook = lambda: _the_hook
    sys.modules["antenv.axon_hooks"] = _mod

bass_utils.upload_artifacts = lambda tmpdir: str(tmpdir)

_MAXW = 1
if not getattr(bass.Bass, "_wait_split_installed", False):
    _orig_to_json_bytes = bass.Bass.to_json_bytes

    def _split_sync_waits(self, *a, **kw):
        bir = json.loads(_orig_to_json_bytes(self, *a, **kw))
        for fn in bir.get("functions", []):
            for blk in fn.get("blocks", []):
                new_insts = []
                for inst in blk.get("instructions", []):
                    si = inst.get("sync_info") or {}
                    waits = si.get("on_wait") or []
                    if len(waits) > _MAXW:
                        extra, keep = waits[:-_MAXW], waits[-_MAXW:]
                        for k in range(0, len(extra), _MAXW):
                            new_insts.append({
                                "debug": inst.get("debug", 0),
                                "engine": inst["engine"],
                                "ins": [], "outs": [],
                                "name": f"{inst['name']}_wsplit{k}",
                                "opcode": "NoOp",
                                "sync_info": {"on_update": [],
                                              "on_wait": extra[k:k + _MAXW]},
                            })
                        si["on_wait"] = keep
                    new_insts.append(inst)
                blk["instructions"] = new_insts
        return json.dumps(bir).encode()

    bass.Bass.to_json_bytes = _split_sync_waits
    bass.Bass._wait_split_installed = True


# ---------------------------------------------------------------------------
# Device kernel
# ---------------------------------------------------------------------------
def _build_nc():
    nc = bass.Bass("TRN2", target_bir_lowering=False, debug=False)
    featd = nc.declare_dram_parameter("featT", [NE, E, BS], FP8, isOutput=False)
    targd = nc.declare_dram_parameter("targT", [NE, E, BS], FP8, isOutput=False)
    w1d = nc.declare_dram_parameter("w1", [E, NE, H], FP8, isOutput=False)
    w2d = nc.declare_dram_parameter("w2", [128, NE, 2, E], FP8, isOutput=False)
    # head = [negI | b1(f32-as-bytes) | W1[e0] | W2[e0]] so one DMA unblocks
    # expert 0 entirely.
    headd = nc.declare_dram_parameter("head", [128, 7, 128], FP8, isOutput=False)
    statsd = nc.declare_dram_parameter("stats", [128, 4 * NE, 6], F32, isOutput=True)

    GE = 8                     # experts per weight-DMA group
    NG = NE // GE

    with tile.TileContext(nc) as tc, contextlib.ExitStack() as ctx:
        wpool = ctx.enter_context(tc.tile_pool(name="weights", bufs=1))
        iopool = ctx.enter_context(tc.tile_pool(name="io", bufs=4))
        hpool = ctx.enter_context(tc.tile_pool(name="hact", bufs=3))
        php = ctx.enter_context(tc.tile_pool(name="ph", bufs=2, space="PSUM"))
        ppp = ctx.enter_context(tc.tile_pool(name="pp", bufs=2, space="PSUM"))

        head_sb = wpool.tile([128, 7, 128], FP8)
        negi_sb = head_sb[:, 0, :]
        b1f = head_sb[:, 1:3, :].bitcast(F32)        # [128, 2, 32] (p, c, n)
        w1e0 = head_sb[:, 3:5, :]                    # [128, 2, 128] (p, c, m)
        w2e0 = head_sb[:, 5:7, :]                    # [128, 2, 128] DR lhsT
        w1_sb = wpool.tile([E, NE, H], FP8)          # [128, 32, 256]
        w2_sb = wpool.tile([128, NE, 2, E], FP8)
        stats_sb = wpool.tile([128, 4 * NE, 6], F32)
        warm_sb = wpool.tile([128, 1], F32)

        # head rides the Activation engine's DGE queue so its descriptor
        # generation runs in parallel with feat0's on SP; the gelu table
        # then loads during the DMA ramp.
        nc.scalar.dma_start(out=head_sb[:], in_=headd[:])
        nc.gpsimd.memset(warm_sb[:], 0.0)
        nc.scalar.activation(warm_sb[:], warm_sb[:],
                             mybir.ActivationFunctionType.Gelu)

        def w1ap(n, c):
            return w1e0[:, c, :] if n == 0 else w1_sb[:, n, ts(c, 128)]

        def w2ap(n):
            return w2e0 if n == 0 else w2_sb[:, n, :, :]

        # flush tile t of the previous expert: mm2 (DoubleRow) + negI + bn
        def flush_tile(pend, t, pair_tile):
            n, hact, targ_sb = pend
            j = t % 2
            nc.tensor.matmul(pair_tile[:, j, :], lhsT=w2ap(n),
                             rhs=hact[:, t, :, :],
                             start=True, stop=False, perf_mode=DR)
            nc.tensor.matmul(pair_tile[:, j, :], lhsT=negi_sb,
                             rhs=targ_sb[:, ts(t, BT)],
                             start=False, stop=True)
            nc.vector.bn_stats(out=stats_sb[:, 4 * n + t, :],
                               in_=pair_tile[:, j, :])

        pending = None
        feat_tiles = {}

        def fetch_feat(n):
            if n < NE and n not in feat_tiles:
                f = iopool.tile([E, BS], FP8, tag="feat", name="feat_sb")
                nc.sync.dma_start(out=f[:], in_=featd[n])
                feat_tiles[n] = f

        fetch_feat(0)
        nc.sync.dma_start(out=w1_sb[:, ts(0, GE), :], in_=w1d[:, ts(0, GE), :])
        # PE warm-up: keep the tensor engine busy through the DMA ramp so the
        # first real matmuls run at full p-state instead of cold-start speed.
        junk_sb = wpool.tile([128, 512], FP8)
        nc.gpsimd.memset(junk_sb[:], 0.0)
        warm_ps = php.tile([128, 2, BT], F32, name="warmps", tag="ph")
        for i in range(10):
            nc.tensor.matmul(warm_ps[:, i % 2, 0:256], lhsT=junk_sb[:, 0:128],
                             rhs=junk_sb[:, 0:256], start=True, stop=True)
        for n in range(NE):
            fetch_feat(n)
            feat_sb = feat_tiles.pop(n)
            fetch_feat(n + 1)
            fetch_feat(n + 2)
            fetch_feat(n + 3)
            targ_sb = iopool.tile([E, BS], FP8, tag="targ")
            nc.sync.dma_start(out=targ_sb[:], in_=targd[n])
            if n < 2 * NG - 1:
                g, which = divmod(n + 1, 2)
                if which == 0:
                    nc.sync.dma_start(out=w1_sb[:, ts(g, GE), :],
                                      in_=w1d[:, ts(g, GE), :])
                else:
                    nc.sync.dma_start(out=w2_sb[:, ts(g, GE), :, :],
                                      in_=w2d[:, ts(g, GE), :, :])
            if n == 25:
                # first 3/4 of the stats is final; overlap the store
                nc.sync.dma_start(out=statsd[:, 0:96, :],
                                  in_=stats_sb[:, 0:96, :])

            hact = hpool.tile([128, NT, 2, BT], FP8)
            p01 = p23 = None
            for g in range(4):
                c, pr = divmod(g, 2)        # chunk, tile-pair of this group
                ph_t = php.tile([128, 2, BT], F32, name=f"ph{g}", tag="ph")
                for i in range(2):
                    nc.tensor.matmul(ph_t[:, i, :], lhsT=w1ap(n, c),
                                     rhs=feat_sb[:, ts(2 * pr + i, BT)],
                                     start=True, stop=True)
                nc.scalar.activation(hact[:, 2 * pr:2 * pr + 2, c, :],
                                     ph_t[:, :, :],
                                     mybir.ActivationFunctionType.Gelu,
                                     bias=b1f[:, c, n:n + 1], scale=ACT_SCALE)
                if pending is not None:
                    if g == 0:
                        p01p = ppp.tile([128, 2, BT], F32, name="p01p", tag="pp")
                        flush_tile(pending, 0, p01p)
                    elif g == 1:
                        flush_tile(pending, 1, p01p)
                    elif g == 2:
                        p23p = ppp.tile([128, 2, BT], F32, name="p23p", tag="pp")
                        flush_tile(pending, 2, p23p)
                    else:
                        flush_tile(pending, 3, p23p)
            pending = (n, hact, targ_sb)
        pf01 = ppp.tile([128, 2, BT], F32, name="pf01", tag="pp")
        for t in (0, 1):
            flush_tile(pending, t, pf01)
        pf23 = ppp.tile([128, 2, BT], F32, name="pf23", tag="pp")
        for t in (2, 3):
            flush_tile(pending, t, pf23)
        nc.sync.dma_start(out=statsd[:, 96:, :], in_=stats_sb[:, 96:, :])
    return nc


LAST_RESULTS = None


def kernel(features, target_features, W1, b1, W2, b2):
    global LAST_RESULTS
    f8 = ml_dtypes.float8_e4m3
    features = np.asarray(features)
    target_features = np.asarray(target_features)
    W1 = np.asarray(W1)
    b1 = np.asarray(b1)
    W2 = np.asarray(W2)
    b2 = np.asarray(b2)

    feat4 = (features * S_X).reshape(C, BS, NE, E).transpose(0, 2, 3, 1).astype(f8)
    targ4 = ((target_features - b2[None]) * S_W2).reshape(C, BS, NE, E) \
        .transpose(0, 2, 3, 1).astype(f8)
    w1h = (W1 * S_W1).transpose(1, 0, 2).astype(f8)                  # [E, NE, H]
    w2h = (W2 * S_W2).reshape(NE, 2, 128, E).transpose(2, 0, 1, 3).astype(f8)
    b1h = np.ascontiguousarray(
        b1.reshape(NE, 2, 128).transpose(2, 1, 0).astype(np.float32))  # [p, c, n]

    negi = (-np.eye(128)).astype(f8)
    head = np.ascontiguousarray(np.concatenate(
        [negi.view(np.uint8),
         b1h.view(np.uint8).reshape(128, 256),
         np.ascontiguousarray(w1h[:, 0, :]).view(np.uint8),
         np.ascontiguousarray(w2h[:, 0, :, :]).reshape(128, 256).view(np.uint8)],
        axis=1)).view(f8).reshape(128, 7, 128)

    nc = _build_nc()
    in_maps = [
        {"featT": np.ascontiguousarray(feat4[c]),
         "targT": np.ascontiguousarray(targ4[c]),
         "w1": w1h, "w2": w2h, "head": head}
        for c in range(C)
    ]
    res = run_bass_kernel_spmd(nc, in_maps, list(range(C)))
    LAST_RESULTS = res
    # stats[p, slot] = [n0, mean0, M2_0, n1, mean1, M2_1] over the two
    # 256-halves of 16*diff; sum(diff^2) = (M2 + n*mean^2) / 256.
    total = 0.0
    for r in res.results:
        st = r["stats"].astype(np.float64)
        total += (st[..., 2] + st[..., 0] * st[..., 1] ** 2
                  + st[..., 5] + st[..., 3] * st[..., 4] ** 2).sum()
    return np.array(total * TTR_SCALE / (B * NE * E), dtype=np.float32)


# revision 31
# speedup vs baseline: 1.0111x; 1.0007x over previous
"""Trainium2 Bass kernel for nn_BaselineDistiller: grouped-expert MLP + MSE loss.

reference:
    h    = einsum('bne,neh->bnh', features, W1) + b1
    g    = gelu(h)                      # exact (erf) gelu
    pred = einsum('bnh,nhe->bne', g, W2) + b2
    out  = mean((pred - target)^2)

Strategy (8 NeuronCores, data-parallel over batch; ~153-157us on HW):
  The kernel is ScalarE(gelu)-bound: 131072 gelu elems/partition/core at
  1 elem/cycle/1.2GHz = 109us floor (+ ~150ns/op overhead). Everything else
  is arranged to keep the ACT engine gap-free and shrink head/tail:
  * All inputs are fp8 e4m3 (feat*8, targ*16 with b2 folded, W1*8, W2*16)
    -> DMA-in halves to ~19MB/core (~57us) and matmuls run at fp8 rate.
  * mm1 (h.T = W1c.T @ feat.T per 128-row chunk) writes PSUM; ACT applies
    gelu with per-chunk bias b1 and scale 1/64 in 4 FD-1024 ops/expert
    (each op is single-chunk so the per-partition bias is uniform), writing
    fp8 hact laid out [128, 4tiles, 2ktiles, 512] (k-tiles contiguous) so
    mm2 runs as ONE fp8 DoubleRow matmul per tile (K=256 at 1 col/cycle —
    2x FLOPs; measured same 216ns as a K=128 matmul).
  * pred tiles accumulate [W2-DR, -I @ (16*(targ-b2))] so PSUM ends holding
    16*diff; one DVE bn_stats per tile (the only DVE reduction needing a
    single PSUM read) yields per-256-group {n, mean, M2}; the host
    reconstructs sum(diff^2) = sum M2 + n*mean^2, dividing out the 16^2.
  * PSUM: ph pool (2 bufs x 2 banks) is a pure mm1->ACT ping-pong; pp pool
    (2 bufs x 2 banks) holds pred pairs. Each expert emits 4 symmetric
    sub-blocks [mm1 pair, ACT, mm2+bn of the PREVIOUS expert's tile k], so
    every cross-engine chain has ~0.5us of slack at the ACT cadence and the
    in-order PE never starves the ACT queue (measured <2us of ACT gaps
    after the DMA ramp; PE ~77%, DVE ~62% of the cadence).
  * Head-ramp engineering: the head DMA's descriptor generation runs on the
    otherwise-idle Activation DGE queue in parallel with feat0's on SP; a
    dummy gelu pulls the 1.3us ACT table load into the DMA ramp; 10 junk
    matmuls keep the PE busy (and at full p-state) until feat0 lands; feat
    DMAs are prefetched 3 experts ahead and the first weight group is
    issued up front so expert 1-2 mm1s never wait; stats ship in a 96/32
    split so the tail only waits on the last expert. Measured ACT gaps:
    ~2.2us (first-data DMA latency) + ~0.2us total elsewhere.
    Typical HW time ~152.5-154us at nominal clocks (the chip DVFS-throttles
    some runs ~20%; the schedule stays gap-free either way).
"""

import contextlib
import ctypes
import json
import sys
import types

import ml_dtypes
import numpy as np

import concourse.bass as bass
import concourse.mybir as mybir
import concourse.tile as tile
from concourse import bass_utils
from concourse.bass import ts
from concourse.bass_utils import run_bass_kernel_spmd

B, NE, E, H = 16384, 32, 128, 256
C = 8              # cores
BS = B // C        # batch rows per core
BT = 512           # batch columns per matmul tile
NT = BS // BT      # 4
FP8 = mybir.dt.float8e4
F32 = mybir.dt.float32
DR = mybir.MatmulPerfMode.DoubleRow

S_X = 8.0          # feature scale into fp8
S_W1 = 8.0
S_W2 = 16.0        # also the target scale (so pred/targ match in PSUM)
ACT_SCALE = 1.0 / (S_X * S_W1)
TTR_SCALE = 1.0 / (S_W2 * S_W2)

# ---------------------------------------------------------------------------
# Environment shims (idempotent):
#  1. antenv.axon_hooks — the image's antenv lacks it; provide the NTFF
#     profile hook via ctypes so trace=True works when a caller requests it.
#  2. upload_artifacts — no bucket access in this container; keep local.
#  3. This walrus build rejects instructions with >1 sync-wait; split the
#     extra waits onto NoOps at BIR-serialization time.
# ---------------------------------------------------------------------------
_AXON_SO = "/opt/axon/libaxon_pjrt.so"


def _make_ntff_hook(so_path):
    try:
        lib = ctypes.CDLL(so_path)
    except OSError:
        return None
    if not hasattr(lib, "axon_start_nrt_profile"):
        return None
    lib.axon_start_nrt_profile.argtypes = [ctypes.POINTER(ctypes.c_int64), ctypes.c_size_t]
    lib.axon_start_nrt_profile.restype = ctypes.c_int64
    lib.axon_stop_nrt_profile.argtypes = [ctypes.c_char_p]
    lib.axon_stop_nrt_profile.restype = ctypes.c_int64

    @contextlib.contextmanager
    def _hook(output_dir, device_ids):
        import jax

        jax.devices()
        if device_ids:
            ids = (ctypes.c_int64 * len(device_ids))(*device_ids)
            rc = lib.axon_start_nrt_profile(ids, len(device_ids))
        else:
            rc = lib.axon_start_nrt_profile(None, 0)
        if rc != 0:
            raise RuntimeError(f"axon_start_nrt_profile rc={rc}")
        try:
            yield
        finally:
            n = lib.axon_stop_nrt_profile(str(output_dir).encode())
            print(f"profile: {n} file(s) written to {output_dir}", file=sys.stderr)

    return _hook


if "antenv.axon_hooks" not in sys.modules:
    _mod = types.ModuleType("antenv.axon_hooks")
    _the_hook = _make_ntff_hook(_AXON_SO)
    _mod.get_axon_ntff_profile_hook = lambda: _the_hook
    sys.modules["antenv.axon_hooks"] = _mod

bass_utils.upload_artifacts = lambda tmpdir: str(tmpdir)

_MAXW = 1
if not getattr(bass.Bass, "_wait_split_installed", False):
    _orig_to_json_bytes = bass.Bass.to_json_bytes

    def _split_sync_waits(self, *a, **kw):
        bir = json.loads(_orig_to_json_bytes(self, *a, **kw))
        for fn in bir.get("functions", []):
            for blk in fn.get("blocks", []):
                new_insts = []
                for inst in blk.get("instructions", []):
                    si = inst.get("sync_info") or {}
                    waits = si.get("on_wait") or []
                    if len(waits) > _MAXW:
                        extra, keep = waits[:-_MAXW], waits[-_MAXW:]
                        for k in range(0, len(extra), _MAXW):
                            new_insts.append({
                                "debug": inst.get("debug", 0),
                                "engine": inst["engine"],
                                "ins": [], "outs": [],
                                "name": f"{inst['name']}_wsplit{k}",
                                "opcode": "NoOp",
                                "sync_info": {"on_update": [],
                                              "on_wait": extra[k:k + _MAXW]},
                            })
                        si["on_wait"] = keep
                    new_insts.append(inst)
                blk["instructions"] = new_insts
        return json.dumps(bir).encode()

    bass.Bass.to_json_bytes = _split_sync_waits
    bass.Bass._wait_split_installed = True


# ---------------------------------------------------------------------------
# Device kernel
# ---------------------------------------------------------------------------
def _build_nc():
    nc = bass.Bass("TRN2", target_bir_lowering=False, debug=False)
    featd = nc.declare_dram_parameter("featT", [NE, E, BS], FP8, isOutput=False)
    targd = nc.declare_dram_parameter("targT", [NE, E, BS], FP8, isOutput=False)
    w1d = nc.declare_dram_parameter("w1", [E, NE, H], FP8, isOutput=False)
    w2d = nc.declare_dram_parameter("w2", [128, NE, 2, E], FP8, isOutput=False)
    # head = [negI | b1(f32-as-bytes) | W1[e0] | W2[e0]] so one DMA unblocks
    # expert 0 entirely.
    headd = nc.declare_dram_parameter("head", [128, 7, 128], FP8, isOutput=False)
    statsd = nc.declare_dram_parameter("stats", [128, 4 * NE, 6], F32, isOutput=True)

    GE = 8                     # experts per weight-DMA group
    NG = NE // GE

    with tile.TileContext(nc) as tc, contextlib.ExitStack() as ctx:
        wpool = ctx.enter_context(tc.tile_pool(name="weights", bufs=1))
        iopool = ctx.enter_context(tc.tile_pool(name="io", bufs=4))
        hpool = ctx.enter_context(tc.tile_pool(name="hact", bufs=3))
        php = ctx.enter_context(tc.tile_pool(name="ph", bufs=2, space="PSUM"))
        ppp = ctx.enter_context(tc.tile_pool(name="pp", bufs=2, space="PSUM"))

        head_sb = wpool.tile([128, 7, 128], FP8)
        negi_sb = head_sb[:, 0, :]
        b1f = head_sb[:, 1:3, :].bitcast(F32)        # [128, 2, 32] (p, c, n)
        w1e0 = head_sb[:, 3:5, :]                    # [128, 2, 128] (p, c, m)
        w2e0 = head_sb[:, 5:7, :]                    # [128, 2, 128] DR lhsT
        w1_sb = wpool.tile([E, NE, H], FP8)          # [128, 32, 256]
        w2_sb = wpool.tile([128, NE, 2, E], FP8)
        stats_sb = wpool.tile([128, 4 * NE, 6], F32)
        warm_sb = wpool.tile([128, 1], F32)

        # head rides the Activation engine's DGE queue so its descriptor
        # generation runs in parallel with feat0's on SP; the gelu table
        # then loads during the DMA ramp.
        nc.scalar.dma_start(out=head_sb[:], in_=headd[:])
        nc.gpsimd.memset(warm_sb[:], 0.0)
        nc.scalar.activation(warm_sb[:], warm_sb[:],
                             mybir.ActivationFunctionType.Gelu)

        def w1ap(n, c):
            return w1e0[:, c, :] if n == 0 else w1_sb[:, n, ts(c, 128)]

        def w2ap(n):
            return w2e0 if n == 0 else w2_sb[:, n, :, :]

        # flush tile t of the previous expert: mm2 (DoubleRow) + negI + bn
        def flush_tile(pend, t, pair_tile):
            n, hact, targ_sb = pend
            j = t % 2
            nc.tensor.matmul(pair_tile[:, j, :], lhsT=w2ap(n),
                             rhs=hact[:, t, :, :],
                             start=True, stop=False, perf_mode=DR)
            nc.tensor.matmul(pair_tile[:, j, :], lhsT=negi_sb,
                             rhs=targ_sb[:, ts(t, BT)],
                             start=False, stop=True)
            nc.vector.bn_stats(out=stats_sb[:, 4 * n + t, :],
                               in_=pair_tile[:, j, :])

        pending = None
        feat_tiles = {}

        def fetch_feat(n):
            if n < NE and n not in feat_tiles:
                f = iopool.tile([E, BS], FP8, tag="feat", name="feat_sb")
                nc.sync.dma_start(out=f[:], in_=featd[n])
                feat_tiles[n] = f

        fetch_feat(0)
        nc.sync.dma_start(out=w1_sb[:, ts(0, GE), :], in_=w1d[:, ts(0, GE), :])
        # PE warm-up: keep the tensor engine busy through the DMA ramp so the
        # first real matmuls run at full p-state instead of cold-start speed.
        junk_sb = wpool.tile([128, 512], FP8)
        nc.gpsimd.memset(junk_sb[:], 0.0)
        warm_ps = php.tile([128, 2, BT], F32, name="warmps", tag="ph")
        for i in range(10):
            nc.tensor.matmul(warm_ps[:, i % 2, 0:256], lhsT=junk_sb[:, 0:128],
                             rhs=junk_sb[:, 0:256], start=True, stop=True)
        for n in range(NE):
            fetch_feat(n)
            feat_sb = feat_tiles.pop(n)
            fetch_feat(n + 1)
            fetch_feat(n + 2)
            fetch_feat(n + 3)
            targ_sb = iopool.tile([E, BS], FP8, tag="targ")
            nc.sync.dma_start(out=targ_sb[:], in_=targd[n])
            if n < 2 * NG - 1:
                g, which = divmod(n + 1, 2)
                if which == 0:
                    nc.sync.dma_start(out=w1_sb[:, ts(g, GE), :],
                                      in_=w1d[:, ts(g, GE), :])
                else:
                    nc.sync.dma_start(out=w2_sb[:, ts(g, GE), :, :],
                                      in_=w2d[:, ts(g, GE), :, :])
            if n == 25:
                # first 3/4 of the stats is final; overlap the store
                nc.sync.dma_start(out=statsd[:, 0:96, :],
                                  in_=stats_sb[:, 0:96, :])

            hact = hpool.tile([128, NT, 2, BT], FP8)
            p01 = p23 = None
            for g in range(4):
                c, pr = divmod(g, 2)        # chunk, tile-pair of this group
                ph_t = php.tile([128, 2, BT], F32, name=f"ph{g}", tag="ph")
                for i in range(2):
                    nc.tensor.matmul(ph_t[:, i, :], lhsT=w1ap(n, c),
                                     rhs=feat_sb[:, ts(2 * pr + i, BT)],
                                     start=True, stop=True)
                nc.scalar.activation(hact[:, 2 * pr:2 * pr + 2, c, :],
                                     ph_t[:, :, :],
                                     mybir.ActivationFunctionType.Gelu,
                                     bias=b1f[:, c, n:n + 1], scale=ACT_SCALE)
                if pending is not None:
                    if g == 0:
                        p01p = ppp.tile([128, 2, BT], F32, name="p01p", tag="pp")
                        flush_tile(pending, 0, p01p)
                    elif g == 1:
                        flush_tile(pending, 1, p01p)
                    elif g == 2:
                        p23p = ppp.tile([128, 2, BT], F32, name="p23p", tag="pp")
                        flush_tile(pending, 2, p23p)
                    else:
                        flush_tile(pending, 3, p23p)
            pending = (n, hact, targ_sb)
        pf01 = ppp.tile([128, 2, BT], F32, name="pf01", tag="pp")
        for t in (0, 1):
            flush_tile(pending, t, pf01)
        pf23 = ppp.tile([128, 2, BT], F32, name="pf23", tag="pp")
        for t in (2, 3):
            flush_tile(pending, t, pf23)
        nc.sync.dma_start(out=statsd[:, 96:, :], in_=stats_sb[:, 96:, :])
    return nc


LAST_RESULTS = None


def kernel(features, target_features, W1, b1, W2, b2):
    global LAST_RESULTS
    f8 = ml_dtypes.float8_e4m3
    features = np.asarray(features)
    target_features = np.asarray(target_features)
    W1 = np.asarray(W1)
    b1 = np.asarray(b1)
    W2 = np.asarray(W2)
    b2 = np.asarray(b2)

    feat4 = (features * S_X).reshape(C, BS, NE, E).transpose(0, 2, 3, 1).astype(f8)
    targ4 = ((target_features - b2[None]) * S_W2).reshape(C, BS, NE, E) \
        .transpose(0, 2, 3, 1).astype(f8)
    w1h = (W1 * S_W1).transpose(1, 0, 2).astype(f8)                  # [E, NE, H]
    w2h = (W2 * S_W2).reshape(NE, 2, 128, E).transpose(2, 0, 1, 3).astype(f8)
    b1h = np.ascontiguousarray(
        b1.reshape(NE, 2, 128).transpose(2, 1, 0).astype(np.float32))  # [p, c, n]

    negi = (-np.eye(128)).astype(f8)
    head = np.ascontiguousarray(np.concatenate(
        [negi.view(np.uint8),
         b1h.view(np.uint8).reshape(128, 256),
         np.ascontiguousarray(w1h[:, 0, :]).view(np.uint8),
         np.ascontiguousarray(w2h[:, 0, :, :]).reshape(128, 256).view(np.uint8)],
        axis=1)).view(f8).reshape(128, 7, 128)

    nc = _build_nc()
    in_maps = [
        {"featT": np.ascontiguousarray(feat4[c]),
         "targT": np.ascontiguousarray(targ4[c]),
         "w1": w1h, "w2": w2h, "head": head}
        for c in range(C)
    ]
    res = run_bass_kernel_spmd(nc, in_maps, list(range(C)))
    LAST_RESULTS = res
    # stats[p, slot] = [n0, mean0, M2_0, n1, mean1, M2_1] over the two
    # 256-halves of 16*diff; sum(diff^2) = (M2 + n*mean^2) / 256.
    total = 0.0
    for r in res.results:
        st = r["stats"].astype(np.float64)
        total += (st[..., 2] + st[..., 0] * st[..., 1] ** 2
                  + st[..., 5] + st[..., 3] * st[..., 4] ** 2).sum()
    return np.array(total * TTR_SCALE / (B * NE * E), dtype=np.float32)
